# revision 1
# baseline (speedup 1.0000x reference)
"""Trainium2 Bass kernel: 8-head attention block (BN-folded projections,
relative-position bias, softmax, GELU + output projection).

Sharding: data-parallel over batch across 8 NeuronCores (2 batch elems/core).
All weights / bias tables replicated; no collectives.

Per-core layout strategy (all "T" tensors are [channel, position]):
  XT  [c=256, n=1024]  bf16   (host pre-transposed, pre-cast)
  QT/KT [d=256, n]     bf16   rows grouped 32/head -> 4 heads per 128-row tile
  V    [n, 512]        bf16   (natural layout, heads contiguous 64-wide)
  dotsT[j, i] psum f32 = sum_d KT[d,j] QT[d,i]  (4 heads packed via row groups)
                       + bias[j,i]/scale^2      (identity-matmul accumulate)
  exp  = ScalarE Exp(scale * psum) -> bf16
  AV: outT[d, i] = sum_j V[j,d] exp[j,i], head pairs packed via PE col groups
      so pair output fills psum partitions 0..127 = gelu tile layout.
  softmax sums via 64-wide all-ones stationary matmuls in the same col
  groups as AV, so each head's sum is replicated across the same 64
  partitions as its AV rows (no cross-partition moves); reciprocal on DVE,
  one fused mul -> gelu input.
  GELU (ScalarE, +BN_v offset as per-partition bias) -> out proj -> +bias -> DMA.

  HW workarounds (device crashes otherwise, found by probing):
  - tile_position (96,0) is fatal (quadrant-3 bug): head 3 runs as a K=64
    matmul at (64,0) against a KT copy with head-2 rows zeroed.
  - tile_position'd matmuls need PSUM-bank-aligned outputs: every packed
    matmul output is a full 2KB bank.
  - exp and gelu are forced into disjoint program phases so the ScalarE
    activation table loads exactly twice.
"""

import os
import numpy as np
import ml_dtypes

import concourse.bass as bass
import concourse.tile as tile
from concourse import bacc, mybir
from concourse.bass_utils import run_bass_kernel_spmd
from concourse.tile import add_dep_helper

NPBF16 = ml_dtypes.bfloat16
BF16 = mybir.dt.bfloat16
F32 = mybir.dt.float32

HEADS, DK, DV = 8, 32, 64
N = 1024          # positions = 32*32
C = 256           # channels
IDV = HEADS * DV  # 512
NCORES = 8
BLOC = 2          # batch elems per core
SCALE = float(DK) ** -0.5
EPS = 1e-5

_CACHE = {}


def _build_nc():
    nc = bacc.Bacc("TRN2", target_bir_lowering=False, debug=False)

    xt_d = nc.declare_dram_parameter("xt", [BLOC, 2, 128, N], BF16, isOutput=False)
    wq_d = nc.declare_dram_parameter("wq", [128, 2, C], BF16, isOutput=False)
    wk_d = nc.declare_dram_parameter("wk", [128, 2, C], BF16, isOutput=False)
    wv_d = nc.declare_dram_parameter("wv", [128, 2, IDV], BF16, isOutput=False)
    wo_d = nc.declare_dram_parameter("wo", [128, 4, C], BF16, isOutput=False)
    oq_d = nc.declare_dram_parameter("oq", [128, 2], F32, isOutput=False)
    ok_d = nc.declare_dram_parameter("ok", [128, 2], F32, isOutput=False)
    ovg_d = nc.declare_dram_parameter("ovg", [128, 4], F32, isOutput=False)
    bout_d = nc.declare_dram_parameter("bout", [128, C], F32, isOutput=False)
    # bias[hg, is, jt, j1, h, i1] = pos_bias[j, i, 4*hg+h] / SCALE^2
    bias_d = nc.declare_dram_parameter("bias", [2, 2, 8, 128, 4, 512], BF16,
                                       isOutput=False)
    id_d = nc.declare_dram_parameter("ident", [128, 128], BF16, isOutput=False)
    out_d = nc.declare_dram_parameter("out", [BLOC, N, C], F32, isOutput=True)

    Exp = mybir.ActivationFunctionType.Exp
    Gelu = mybir.ActivationFunctionType.Gelu

    with tile.TileContext(nc) as tc:
        with (
            tc.tile_pool(name="const", bufs=1) as const,
            tc.tile_pool(name="persist", bufs=1) as persist,
            tc.tile_pool(name="biasp", bufs=12) as biasp,
            tc.tile_pool(name="expp", bufs=10) as expp,
            tc.tile_pool(name="recp", bufs=3) as recp,
            tc.tile_pool(name="outp", bufs=4) as outp,
            tc.tile_pool(name="dpsum", bufs=3, space="PSUM") as dpsum,
            tc.tile_pool(name="avpsum", bufs=1, space="PSUM") as avpsum,
        ):
            dma = nc.sync

            # ---------------- constants ----------------
            wq_s = const.tile([128, 2, C], BF16, tag="wq")
            dma.dma_start(wq_s[:], wq_d[:])
            wk_s = const.tile([128, 2, C], BF16, tag="wk")
            dma.dma_start(wk_s[:], wk_d[:])
            wv_s = const.tile([128, 2, IDV], BF16, tag="wv")
            dma.dma_start(wv_s[:], wv_d[:])
            wo_s = const.tile([128, 4, C], BF16, tag="wo")
            dma.dma_start(wo_s[:], wo_d[:])
            oq_s = const.tile([128, 2], F32, tag="oq")
            dma.dma_start(oq_s[:], oq_d[:])
            ok_s = const.tile([128, 2], F32, tag="ok")
            dma.dma_start(ok_s[:], ok_d[:])
            ovg_s = const.tile([128, 4], F32, tag="ovg")
            dma.dma_start(ovg_s[:], ovg_d[:])
            bout_s = const.tile([128, C], F32, tag="bout")
            dma.dma_start(bout_s[:], bout_d[:])
            ident_s = const.tile([128, 128], BF16, tag="ident")
            dma.dma_start(ident_s[:], id_d[:])
            ones_s = const.tile([128, 64], BF16, tag="ones")
            nc.vector.memset(ones_s[:], 1.0)

            # ---------------- load x (pre-transposed on host) ----------------
            xt = {}
            for b in range(BLOC):
                for ct in range(2):
                    t = persist.tile([128, N], BF16, tag=f"xt{b}{ct}", name=f"xt{b}{ct}")
                    dma.dma_start(t[:], xt_d[b, ct])
                    xt[b, ct] = t

            # ---------------- Q/K projections -> QT/KT [d, i] bf16 ----------
            qt, kt, kzt = {}, {}, {}
            for b in range(BLOC):
                for tt in range(2):  # 128-row d-tile == head group tt
                    qtile = persist.tile([128, N], BF16, tag=f"qt{b}{tt}", name=f"qt{b}{tt}")
                    ktile = persist.tile([128, N], BF16, tag=f"kt{b}{tt}", name=f"kt{b}{tt}")
                    qt[b, tt], kt[b, tt] = qtile, ktile
                    # kz: copy of KT with head-2 rows zeroed; lets head 3 run
                    # as a K=64 matmul at row group 2 (tile_position (96,0)
                    # crashes this runtime: quadrant-3 HW bug)
                    kz = persist.tile([128, N], BF16, tag=f"kz{b}{tt}",
                                      name=f"kz{b}{tt}")
                    kzt[b, tt] = kz
                    nc.vector.memset(kz[64:96, :], 0.0)
                    for wsb, osb, dst in ((wq_s, oq_s, qtile), (wk_s, ok_s, ktile)):
                        for i2 in range(2):
                            ps = dpsum.tile([128, 512], F32, tag="dps")
                            for ct in range(2):
                                nc.tensor.matmul(
                                    ps[:],
                                    wsb[:, ct, tt * 128:(tt + 1) * 128],
                                    xt[b, ct][:, i2 * 512:(i2 + 1) * 512],
                                    start=(ct == 0), stop=(ct == 1),
                                )
                            nc.vector.tensor_scalar_add(
                                dst[:, i2 * 512:(i2 + 1) * 512], ps[:],
                                osb[:, tt:tt + 1])
                            if dst is ktile:
                                nc.vector.tensor_scalar_add(
                                    kz[96:128, i2 * 512:(i2 + 1) * 512],
                                    ps[96:128, :], osb[96:128, tt:tt + 1])

            # ---------------- V projection -> V [j, (h d)] bf16 --------------
            vt = {}
            for b in range(BLOC):
                for it in range(8):
                    v = persist.tile([128, 8, DV], BF16, tag=f"v{b}{it}", name=f"v{b}{it}")
                    vt[b, it] = v
                    ps = dpsum.tile([128, 512], F32, tag="dps")
                    for ct in range(2):
                        nc.tensor.matmul(
                            ps[:],
                            xt[b, ct][:, it * 128:(it + 1) * 128],
                            wv_s[:, ct, :],
                            start=(ct == 0), stop=(ct == 1),
                        )
                    nc.vector.tensor_copy(v[:, :, :], ps[:].rearrange("p (h d) -> p h d", h=8))

            # gelu input tiles [128 = head pair, 1024] bf16, per (b, dt)
            gelu_t = {}
            for b in range(BLOC):
                for dt in range(4):
                    gelu_t[b, dt] = persist.tile([128, N], BF16, tag=f"g{b}{dt}", name=f"g{b}{dt}")

            last_exp = [None]
            # ---------------- attention ----------------
            for hg in range(2):          # head group of 4 (row-packed dots)
                for isl in range(2):     # i slice of 512
                    bias_t = {}
                    for jt in range(8):
                        bt = biasp.tile([128, 4, 512], BF16, tag="bias", name=f"bias{hg}{isl}{jt}")
                        dma.dma_start(bt[:], bias_d[hg, isl, jt])
                        bias_t[jt] = bt
                    for b in range(BLOC):
                        # --- dots + bias + exp over all j tiles ---
                        # tile_position'd matmul outputs must be PSUM
                        # bank-aligned (half-bank offsets crash the device),
                        # so each head gets a full 512-wide bank.
                        exp_t = {}
                        i0 = isl * 512
                        for jt in range(8):
                            et = expp.tile([128, 4, 512], BF16, tag="exp", name=f"exp{b}{jt}")
                            exp_t[jt] = et
                            dtiles = []
                            for pair in range(2):
                                dps = dpsum.tile([128, 2, 512], F32, tag="dps")
                                dtiles.append(dps)
                                for half in range(2):
                                    h = 2 * pair + half
                                    if h < 3:
                                        nc.tensor.matmul(
                                            dps[:, half, :],
                                            kt[b, hg][32 * h:32 * h + 32,
                                                      jt * 128:(jt + 1) * 128],
                                            qt[b, hg][32 * h:32 * h + 32,
                                                      i0:i0 + 512],
                                            start=True, stop=False,
                                            tile_position=(32 * h, 0),
                                        )
                                    else:
                                        # head 3: K=64 at row group 2 with
                                        # head-2 weight rows zeroed
                                        # (tile_position (96,0) is broken)
                                        nc.tensor.matmul(
                                            dps[:, half, :],
                                            kzt[b, hg][64:128,
                                                       jt * 128:(jt + 1) * 128],
                                            qt[b, hg][64:128, i0:i0 + 512],
                                            start=True, stop=False,
                                            tile_position=(64, 0),
                                        )
                            # all bias matmuls together: identity stationary
                            # loads once per j tile
                            for pair in range(2):
                                for half in range(2):
                                    h = 2 * pair + half
                                    nc.tensor.matmul(
                                        dtiles[pair][:, half, :],
                                        ident_s[:],
                                        bias_t[jt][:, h, :],
                                        start=False, stop=True,
                                    )
                            for pair in range(2):
                                ae = nc.scalar.activation(
                                    et[:, 2 * pair:2 * pair + 2, :],
                                    dtiles[pair][:], Exp, scale=SCALE)
                                last_exp[0] = ae
                        # --- AV + softmax sums, head pairs in col groups ---
                        for p in range(2):
                            av = avpsum.tile([128, 512], F32, tag="av")
                            sums = avpsum.tile([128, 512], F32, tag="sums")
                            for half in range(2):
                                h = 2 * p + half
                                hglob = 4 * hg + h
                                for jt in range(8):
                                    nc.tensor.matmul(
                                        av[64 * half:64 * half + 64, :],
                                        vt[b, jt][:, hglob, :],
                                        exp_t[jt][:, h, :],
                                        start=(jt == 0), stop=(jt == 7),
                                        tile_position=(0, 64 * half),
                                    )
                            # ones-stationary sums after av: ones load once,
                            # replicated sum lands on the same partitions as av
                            for half in range(2):
                                h = 2 * p + half
                                for jt in range(8):
                                    nc.tensor.matmul(
                                        sums[64 * half:64 * half + 64, :],
                                        ones_s[:],
                                        exp_t[jt][:, h, :],
                                        start=(jt == 0), stop=(jt == 7),
                                        tile_position=(0, 64 * half),
                                    )
                            rec = recp.tile([128, 512], F32, tag="rec")
                            nc.vector.reciprocal(rec[:], sums[:])
                            dt = 2 * hg + p
                            nc.vector.tensor_mul(
                                gelu_t[b, dt][:, isl * 512:(isl + 1) * 512],
                                av[:], rec[:])

            # ---------------- GELU + output projection ----------------
            for b in range(BLOC):
                for dt in range(4):
                    gi = nc.scalar.activation(gelu_t[b, dt][:], gelu_t[b, dt][:],
                                              Gelu, bias=ovg_s[:, dt:dt + 1],
                                              scale=1.0)
                    if last_exp[0] is not None:
                        add_dep_helper(gi.ins, last_exp[0].ins, sync=False,
                                       reason="group ACT table sets")
            for b in range(BLOC):
                for it in range(8):
                    ops = avpsum.tile([128, C], F32, tag="sums")
                    for dt in range(4):
                        nc.tensor.matmul(
                            ops[:],
                            gelu_t[b, dt][:, it * 128:(it + 1) * 128],
                            wo_s[:, dt, :],
                            start=(dt == 0), stop=(dt == 3),
                        )
                    osb = outp.tile([128, C], F32, tag="osb")
                    nc.vector.tensor_add(osb[:], ops[:], bout_s[:])
                    dma.dma_start(out_d[b, it * 128:(it + 1) * 128, :], osb[:])

    nc.compile()
    return nc


def _host_prep(x, w_q, bn_q, w_k, bn_k, w_v, bn_v, w_out, b_out, bn_out,
               pos_table):
    """Fold BN into weights, build bias table, shard across cores."""
    def fold(bn):
        g, b_, m, v = [np.asarray(a, np.float64) for a in bn]
        s = g / np.sqrt(v + EPS)
        return s, b_ - m * s

    sq, oq = fold(bn_q)
    sk, ok = fold(bn_k)
    sv, ov = fold(bn_v)
    so, oo = fold(bn_out)

    def wtile(w, s, ncols):
        # [C_in, D] * s[D] -> [128, C_in//128, D] bf16 (partition-major)
        w_eff = (np.asarray(w, np.float64) * s[None, :]).astype(np.float32)
        return np.ascontiguousarray(
            w_eff.reshape(-1, 128, ncols).transpose(1, 0, 2)).astype(NPBF16)

    wq = wtile(w_q, sq, C)
    wk = wtile(w_k, sk, C)
    wv = wtile(w_v, sv, IDV)
    wo = wtile(w_out, so, C)

    oq_t = np.ascontiguousarray(oq.astype(np.float32).reshape(2, 128).T)
    ok_t = np.ascontiguousarray(ok.astype(np.float32).reshape(2, 128).T)
    ovg_t = np.ascontiguousarray(ov.astype(np.float32).reshape(4, 128).T)
    bout_eff = (np.asarray(b_out, np.float64) * so + oo).astype(np.float32)
    bout_t = np.ascontiguousarray(np.broadcast_to(bout_eff, (128, C)))

    # position bias table
    r = np.arange(32)
    pos = np.stack(np.meshgrid(r, r, indexing="ij"), axis=-1).reshape(-1, 2)
    rel = np.abs(pos[:, None, :] - pos[None, :, :])
    idx = rel[..., 0] * 32 + rel[..., 1]           # [n, n]
    bias = np.asarray(pos_table, np.float32)[idx]  # [j, i, 8]
    bias = bias / (SCALE * SCALE)
    # -> [hg, is, jt, j1, h, i1]
    bias = bias.reshape(8, 128, 2, 512, 2, 4)      # jt, j1, is, i1, hg, h
    bias = np.ascontiguousarray(
        bias.transpose(4, 2, 0, 1, 5, 3)).astype(NPBF16)

    ident = np.eye(128, dtype=NPBF16)

    x = np.asarray(x, np.float32).reshape(-1, N, C)      # [B, n, C]
    common = dict(wq=wq, wk=wk, wv=wv, wo=wo, oq=oq_t, ok=ok_t, ovg=ovg_t,
                  bout=bout_t, bias=bias, ident=ident)
    in_maps = []
    for c in range(NCORES):
        xl = x[c * BLOC:(c + 1) * BLOC]                  # [2, n, C]
        xtl = xl.transpose(0, 2, 1).reshape(BLOC, 2, 128, N).astype(NPBF16)
        in_maps.append(dict(common, xt=np.ascontiguousarray(xtl)))
    return in_maps


def kernel(**inputs):
    if "nc" not in _CACHE:
        _CACHE["nc"] = _build_nc()
    nc = _CACHE["nc"]
    in_maps = _host_prep(**inputs)
    res = run_bass_kernel_spmd(nc, in_maps, core_ids=list(range(NCORES)),
                               trace=bool(int(os.environ.get("KTRACE", "0"))))
    _CACHE["last_result"] = res
    outs = [res.results[c]["out"].reshape(BLOC, 32, 32, C)
            for c in range(NCORES)]
    return np.concatenate(outs, axis=0).astype(np.float32)


if __name__ == "__main__":
    nc = _build_nc()
    print("build + compile OK")

